# revision 6
# baseline (speedup 1.0000x reference)
"""Trainium2 Bass kernel for nn_Conv_39273180955616.

Computes, for X:(16,64,512,512) f32, K:(1,1,7,7), b:(1,1,1,1):
    out[n,c] = correlate2d(X[n,c], Keff, pad=3) + 49*b
where Keff = K.sum(axis=(0,1)).

Strategy: pure data parallel over the 1024 (n,c) planes -> 128 planes/core
on 8 cores.  Per plane, the 7x7 correlation runs on TensorE as
banded-Toeplitz matmuls: the h-dimension contraction is a [K<=128, 128]
band matrix (7 diagonals of one kernel column) against an image block
(rows on partitions), and the 7 w-shifts are free-dim offsets into a
zero-padded (W+6) image row, accumulated in PSUM.  The 24-row bottom
tiles of 4 consecutive planes are packed into one block-diagonal matmul
set (stacked on partitions), cutting the matmul count by 15%.

DMA layout: the host pre-swizzles each plane's 4 input row-blocks into a
partition-major [128, 4*WPAD] bf16 layout so each plane loads with ONE
dma_start of 128 contiguous 4.1KB descriptors (SP HWDGE ring); the group
bottom block is pre-packed block-diagonally ([108, WPAD], ACT ring).
Outputs are evicted from PSUM as fp16 (bias added during eviction,
alternating ScalarE/VectorE) and stored via SWDGE so descriptors spread
across all 16 SDMA engines; host upcasts to f32.
"""
import numpy as np
import ml_dtypes

import concourse.bass as bass
import concourse.tile as tile
from concourse import bacc, mybir
from concourse.bass_utils import run_bass_kernel_spmd

N_CORES = 8
H = 512
W = 512
WPAD = W + 6  # 3 zero columns each side
N_PLANES_TOTAL = 16 * 64
PLANES_PER_CORE = N_PLANES_TOTAL // N_CORES  # 128
GROUP = 4  # planes per bottom-tile merge group
BSTARTS = (0, 119, 241, 363)  # input row start of each main block

# Per-plane tiles: 4 x 122 output rows (kinds 0/1); the 24-row bottom
# tile (kind 2) is handled once per GROUP planes as a block-diagonal
# [108, 96] matmul (4 x K=27 / M=24 blocks stacked on partitions).
# (out_row0, out_rows, in_row0, in_rows, kind)
TILES = [
    (0, 122, 0, 125, 0),
    (122, 122, 119, 128, 1),
    (244, 122, 241, 128, 1),
    (366, 122, 363, 128, 1),
]
KIND_K = {0: 125, 1: 128, 2: GROUP * 27}
M_PAD = 128  # lhsT padded to 128 cols -> FWL eligible; pad rows are zero
WCOLS = 3 * 7 * M_PAD


def _build_weight_pack(Keff: np.ndarray) -> np.ndarray:
    """Keff (7,7) f32 -> packed banded-Toeplitz lhsT matrices [128, WCOLS] bf16.

    Matrix for (kind, dw) sits at cols [(kind*7+dw)*128, ...+128).
    lhsT[p, m] = Keff[dh, dw], dh = p - m (+3 for kind 0); matmul computes
    out[m, w] = sum_p lhsT[p, m] * block[p, w + dw].  Kind 2 is the
    block-diagonal stack of GROUP bottom tiles: block g at rows
    [27g, 27g+27) x cols [24g, 24g+24).
    """
    wp = np.zeros((128, WCOLS), np.float32)
    for kind in (0, 1):
        Kk = KIND_K[kind]
        p = np.arange(Kk)[:, None]
        m = np.arange(122)[None, :]
        dh = p - m + (3 if kind == 0 else 0)
        ok = (dh >= 0) & (dh < 7)
        for dw in range(7):
            mat = np.zeros((Kk, M_PAD), np.float32)
            mat[:, :122][ok] = Keff[dh[ok], dw]
            c0 = (kind * 7 + dw) * M_PAD
            wp[:Kk, c0:c0 + M_PAD] = mat
    # kind 2 block-diagonal
    p = np.arange(27)[:, None]
    m = np.arange(24)[None, :]
    dh = p - m
    ok = (dh >= 0) & (dh < 7)
    for dw in range(7):
        blk = np.zeros((27, 24), np.float32)
        blk[ok] = Keff[dh[ok], dw]
        c0 = (2 * 7 + dw) * M_PAD
        for g in range(GROUP):
            wp[27 * g:27 * g + 27, c0 + 24 * g:c0 + 24 * g + 24] = blk
    return wp.astype(ml_dtypes.bfloat16)


_NC_CACHE = {}


def _get_module(n_planes: int):
    if n_planes in _NC_CACHE:
        return _NC_CACHE[n_planes]
    assert n_planes % GROUP == 0
    nc = bacc.Bacc("TRN2", target_bir_lowering=False, debug=False,
                   num_devices=N_CORES)
    xp = nc.dram_tensor("xp", [n_planes, 128, 4 * WPAD], mybir.dt.bfloat16,
                        kind="ExternalInput")
    xg_d = nc.dram_tensor("xg", [n_planes // GROUP, GROUP * 27, WPAD],
                          mybir.dt.bfloat16, kind="ExternalInput")
    wt = nc.dram_tensor("wt", [128, WCOLS], mybir.dt.bfloat16,
                        kind="ExternalInput")
    bv = nc.dram_tensor("bv", [128, 1], mybir.dt.float32,
                        kind="ExternalInput")
    out = nc.dram_tensor("out", [n_planes, H, W], mybir.dt.float16,
                         kind="ExternalOutput")

    x_elems = 128 * 4 * WPAD  # per-plane element count in xp
    g_elems = GROUP * 27 * WPAD

    with tile.TileContext(nc) as tc:
        with (
            tc.tile_pool(name="wp", bufs=1) as wpool,
            tc.tile_pool(name="xa", bufs=8) as xapool,
            tc.tile_pool(name="xg", bufs=3) as xgpool,
            tc.tile_pool(name="ps", bufs=8, space="PSUM") as pspool,
            tc.tile_pool(name="ob", bufs=10) as obpool,
            tc.tile_pool(name="og", bufs=3) as ogpool,
        ):
            wtile = wpool.tile([128, WCOLS], mybir.dt.bfloat16)
            nc.sync.dma_start(wtile[:], wt.ap())
            btile = wpool.tile([128, 1], mybir.dt.float32)
            nc.sync.dma_start(btile[:], bv.ap())

            def evict(engine, dst, src, rows):
                if engine == "act":
                    nc.scalar.activation(
                        dst, src, mybir.ActivationFunctionType.Identity,
                        bias=btile[:rows, :], scale=1.0)
                else:
                    nc.vector.tensor_scalar_add(dst, src, btile[:rows, :])

            for g0 in range(0, n_planes, GROUP):
                # bottom rows (485..511) of GROUP planes, pre-packed
                # block-diagonally on host; one load on the ACT ring
                xg = xgpool.tile([GROUP * 27, WPAD], mybir.dt.bfloat16)
                nc.scalar.dma_start(
                    xg[:], bass.AP(xg_d, (g0 // GROUP) * g_elems,
                                   [[WPAD, GROUP * 27], [1, WPAD]]))
                for p in range(g0, g0 + GROUP):
                    # one partition-major load per plane (SP ring):
                    # partition r holds rows (r, 119+r, 241+r, 363+r)
                    xa = xapool.tile([128, 4 * WPAD], mybir.dt.bfloat16)
                    nc.sync.dma_start(
                        xa[:], bass.AP(xp, p * x_elems,
                                       [[4 * WPAD, 128], [1, 4 * WPAD]]))

                    ob = obpool.tile([122, 4 * W], mybir.dt.float16)
                    for t, (or0, oh, ir0, ih, kind) in enumerate(TILES):
                        pt = pspool.tile([128, W], mybir.dt.float32)
                        for dw in range(7):
                            c0 = (kind * 7 + dw) * M_PAD
                            nc.tensor.matmul(
                                pt[:, :], wtile[:ih, c0:c0 + M_PAD],
                                xa[:ih, t * WPAD + dw:t * WPAD + dw + W],
                                start=(dw == 0), stop=(dw == 6))
                        evict("act" if t % 2 == 0 else "dve",
                              ob[:, t * W:(t + 1) * W], pt[:122, :], 122)
                    # rows 0..487 = 4 tiles of 122 (fp16); alternate SWDGE
                    # (spreads over all 16 SDMA engines) with the SP HWDGE
                    # ring to halve the SWDGE Q7 descriptor-gen backlog
                    store_eng = nc.gpsimd if p % 2 == 0 else nc.sync
                    store_eng.dma_start(
                        bass.AP(out, p * H * W,
                                [[W, 122], [122 * W, 4], [1, W]]),
                        ob[:].rearrange("p (b w) -> p b w", b=4))

                # ---- merged bottom tiles of the group ----
                pt = pspool.tile([128, W], mybir.dt.float32)
                for dw in range(7):
                    c0 = (2 * 7 + dw) * M_PAD
                    nc.tensor.matmul(
                        pt[:, :], wtile[:GROUP * 27, c0:c0 + M_PAD],
                        xg[:, dw:dw + W], start=(dw == 0), stop=(dw == 6))
                og = ogpool.tile([GROUP * 24, W], mybir.dt.float16)
                evict("act", og[:], pt[:GROUP * 24, :], GROUP * 24)
                for g in range(GROUP):
                    nc.gpsimd.dma_start(
                        bass.AP(out, ((g0 + g) * H + 488) * W,
                                [[W, 24], [1, W]]),
                        og[24 * g:24 * g + 24, :])

    nc.compile()
    _NC_CACHE[n_planes] = nc
    return nc


def _prep_inputs(X, K, b, n_cores=N_CORES):
    Keff = np.asarray(K, np.float32).sum(axis=(0, 1))
    wt = _build_weight_pack(Keff)
    bias = np.float32(np.asarray(b).reshape(-1)[0]) * np.float32(K.size)
    bv = np.full((128, 1), bias, np.float32)

    Xr = np.asarray(X, np.float32).reshape(-1, H, W)
    n_total = Xr.shape[0]
    per = n_total // n_cores
    # zero-padded bf16 planes, then swizzled into partition-major blocks
    Xpad = np.zeros((n_total, H, WPAD), ml_dtypes.bfloat16)
    Xpad[:, :, 3:3 + W] = Xr.astype(ml_dtypes.bfloat16)
    Xp = np.empty((n_total, 128, 4, WPAD), ml_dtypes.bfloat16)
    for bi, s in enumerate(BSTARTS):
        Xp[:, :, bi, :] = Xpad[:, s:s + 128, :]
    Xp = Xp.reshape(n_total, 128, 4 * WPAD)
    # group bottom blocks: [group, 4*27, WPAD], plane g at partitions 27g..
    Xg = np.ascontiguousarray(
        Xpad[:, 485:512, :].reshape(n_total // GROUP, GROUP * 27, WPAD))
    in_maps = [
        {"xp": Xp[i * per:(i + 1) * per],
         "xg": Xg[i * (per // GROUP):(i + 1) * (per // GROUP)],
         "wt": wt, "bv": bv}
        for i in range(n_cores)
    ]
    return in_maps, per


def kernel(X, K, b):
    in_maps, per = _prep_inputs(X, K, b)
    nc = _get_module(per)
    res = run_bass_kernel_spmd(nc, in_maps, list(range(N_CORES)))
    out = np.concatenate([res.results[i]["out"] for i in range(N_CORES)],
                         axis=0).astype(np.float32)
    return out.reshape(np.asarray(X).shape)


# revision 15
# speedup vs baseline: 1.2418x; 1.2418x over previous
"""Trainium2 Bass kernel for nn_Conv_39273180955616.

Computes, for X:(16,64,512,512) f32, K:(1,1,7,7), b:(1,1,1,1):
    out[n,c] = correlate2d(X[n,c], Keff, pad=3) + 49*b
where Keff = K.sum(axis=(0,1)).

Strategy: pure data parallel over the 1024 (n,c) planes -> 128 planes/core
on 8 cores.  Per plane, the 7x7 correlation runs on TensorE as
banded-Toeplitz matmuls in fp8(e4m3) DoubleRow mode: the h-dimension
contraction is a [128, 128] band matrix (7 diagonals of one kernel
column, dh = p - m) against an image block (rows on partitions), and the
7 w-shifts are free-dim offsets into a zero-padded image row.  DoubleRow
packs TWO w-shifts per matmul (2 fp8 MACs/cell/cycle): the rhs is a
3-dim AP [K, 2, W] whose pair dim selects between the image (slot 0) and
a one-column-left-shifted copy (slot 1, pair stride 2080 -- the HW
requires a multiple of 16), so 7 shifts take 4 matmuls: pairs
(0,1),(2,3),(4,5),(5,6), the last with zeroed slot-0 weights.  The host
pre-pads 3 zero rows into block 0 so all 4 row-blocks share one weight
kind.  The 24-row bottom tiles of 4 consecutive planes are packed into
one block-diagonal matmul set (stacked on partitions).

DMA: one partition-major fp8 load per plane ([128, 4160], SP ring,
contiguous 4KB descriptors); group bottom blocks on the ACT ring.
PSUM is evicted as fp16 with the bias added (alternating ScalarE /
VectorE) and stored via SWDGE so descriptors spread across all 16 SDMA
engines; the host upcasts to f32.
"""
import numpy as np
import ml_dtypes

import concourse.bass as bass
import concourse.tile as tile
from concourse import bacc, mybir
from concourse.bass_utils import run_bass_kernel_spmd

N_CORES = 8
H = 512
W = 512
WPAD = W + 6   # 3 zero columns each side (valid data span)
WB = 520       # per-block width in the swizzled layout (padded)
SLOT = 4 * WB  # 2080, pair stride for main tiles (multiple of 16)
GWB = 528      # xg per-slot width (multiple of 16)
N_PLANES_TOTAL = 16 * 64
PLANES_PER_CORE = N_PLANES_TOTAL // N_CORES  # 128
GROUP = 4  # planes per bottom-tile merge group
# input row start of blocks 1..3; block 0 is [3 zero rows, rows 0..124]
BSTARTS = (119, 241, 363)
# DoubleRow pairs: (rhs base offset, slot0 dw or None, slot1 dw=off+1)
PAIRS = [(0, 0, 1), (2, 2, 3), (4, 4, 5), (5, None, 6)]
KM = 122   # output rows per main tile
M_PAD = 128
WCOLS = 8 * 2 * M_PAD  # 4 main pairs + 4 group pairs, 2 slots each

FP8 = ml_dtypes.float8_e4m3


def _band(Keff, Kk, M, dw):
    """[Kk, M_PAD] band matrix: mat[p, m] = Keff[p - m, dw]."""
    mat = np.zeros((Kk, M_PAD), np.float32)
    p = np.arange(Kk)[:, None]
    m = np.arange(M)[None, :]
    dh = p - m
    ok = (dh >= 0) & (dh < 7)
    sub = np.zeros((Kk, M), np.float32)
    sub[ok] = Keff[dh[ok], dw]
    mat[:, :M] = sub
    return mat


def _build_weight_pack(K8: np.ndarray) -> np.ndarray:
    """K8 (7,7) f32 (already e4m3-rounded) -> [128, WCOLS] fp8 lhsT pairs.

    Main pair j at cols [j*256, +256): slot0 [0:128], slot1 [128:256].
    Group (block-diagonal, GROUP bottom tiles) pair j at [(4+j)*256, ...).
    """
    wp = np.zeros((128, WCOLS), np.float32)
    for j, (_, dw0, dw1) in enumerate(PAIRS):
        c0 = j * 2 * M_PAD
        if dw0 is not None:
            wp[:, c0:c0 + M_PAD] = _band(K8, 128, KM, dw0)
        wp[:, c0 + M_PAD:c0 + 2 * M_PAD] = _band(K8, 128, KM, dw1)
    # group kind: block-diagonal stack of GROUP (27 -> 24) bottom bands
    for j, (_, dw0, dw1) in enumerate(PAIRS):
        c0 = (4 + j) * 2 * M_PAD
        for s, dw in ((0, dw0), (1, dw1)):
            if dw is None:
                continue
            blk = _band(K8, 27, 24, dw)[:, :24]
            for g in range(GROUP):
                wp[27 * g:27 * g + 27,
                   c0 + s * M_PAD + 24 * g:c0 + s * M_PAD + 24 * g + 24] = blk
    return wp.astype(FP8)


_NC_CACHE = {}


def _get_module(n_planes: int):
    if n_planes in _NC_CACHE:
        return _NC_CACHE[n_planes]
    assert n_planes % GROUP == 0
    nc = bacc.Bacc("TRN2", target_bir_lowering=False, debug=False,
                   num_devices=N_CORES)
    xp = nc.dram_tensor("xp", [n_planes, 128, 2 * SLOT], mybir.dt.float8e4,
                        kind="ExternalInput")
    xg_d = nc.dram_tensor("xg", [n_planes // GROUP, GROUP * 27, 2 * GWB],
                          mybir.dt.float8e4, kind="ExternalInput")
    wt = nc.dram_tensor("wt", [128, WCOLS], mybir.dt.float8e4,
                        kind="ExternalInput")
    bv = nc.dram_tensor("bv", [128, 1], mybir.dt.float32,
                        kind="ExternalInput")
    out = nc.dram_tensor("out", [n_planes, H, W], mybir.dt.float16,
                         kind="ExternalOutput")

    x_elems = 128 * 2 * SLOT
    g_elems = GROUP * 27 * 2 * GWB
    DR = mybir.MatmulPerfMode.DoubleRow

    with tile.TileContext(nc) as tc:
        with (
            tc.tile_pool(name="wp", bufs=1) as wpool,
            tc.tile_pool(name="xa", bufs=8) as xapool,
            tc.tile_pool(name="xg", bufs=3) as xgpool,
            tc.tile_pool(name="ps", bufs=8, space="PSUM") as pspool,
            tc.tile_pool(name="ob", bufs=10) as obpool,
            tc.tile_pool(name="og", bufs=3) as ogpool,
        ):
            wtile = wpool.tile([128, WCOLS], mybir.dt.float8e4)
            nc.sync.dma_start(wtile[:], wt.ap())
            btile = wpool.tile([128, 1], mybir.dt.float32)
            nc.sync.dma_start(btile[:], bv.ap())

            def evict(engine, dst, src, rows):
                if engine == "act":
                    nc.scalar.activation(
                        dst, src, mybir.ActivationFunctionType.Identity,
                        bias=btile[:rows, :], scale=1.0)
                else:
                    nc.vector.tensor_scalar_add(dst, src, btile[:rows, :])

            def dr_matmul(pt, wcol0, tens, pstride, sstride, rhs_off, Kk, j):
                """One DoubleRow matmul: lhsT [K,2,128], rhs [K,2,512].

                rhs pair dim selects image slot 0 / shifted slot 1
                (sstride must be a multiple of 16 for DoubleRow).
                """
                lhsT = wtile[:Kk, wcol0:wcol0 + 2 * M_PAD].rearrange(
                    "k (two m) -> k two m", two=2)
                rhs = bass.AP(tens, rhs_off,
                              [[pstride, Kk], [sstride, 2], [1, W]])
                nc.tensor.matmul(pt[:, :], lhsT, rhs, start=(j == 0),
                                 stop=(j == 3), perf_mode=DR)

            for g0 in range(0, n_planes, GROUP):
                # bottom rows (485..511) of GROUP planes, pre-packed
                # block-diagonally on host; one load on the ACT ring
                xg = xgpool.tile([GROUP * 27, 2 * GWB], mybir.dt.float8e4)
                nc.scalar.dma_start(
                    xg[:], bass.AP(xg_d, (g0 // GROUP) * g_elems,
                                   [[2 * GWB, GROUP * 27], [1, 2 * GWB]]))
                for p in range(g0, g0 + GROUP):
                    # one partition-major load per plane (SP ring):
                    # partition r holds [4 blocks | 4 shifted blocks]
                    xa = xapool.tile([128, 2 * SLOT], mybir.dt.float8e4)
                    nc.sync.dma_start(
                        xa[:], bass.AP(xp, p * x_elems,
                                       [[2 * SLOT, 128], [1, 2 * SLOT]]))

                    ob = obpool.tile([122, 4 * W], mybir.dt.float16)
                    pts = [pspool.tile([128, W], mybir.dt.float32,
                                       name="pt")
                           for t in range(4)]
                    xat = xa[:].tensor
                    for j, (off, dw0, dw1) in enumerate(PAIRS):
                        for t in range(4):
                            dr_matmul(pts[t], j * 2 * M_PAD, xat, 2 * SLOT,
                                      SLOT, t * WB + off, 128, j)
                    for t in range(4):
                        evict("act" if t % 2 == 0 else "dve",
                              ob[:, t * W:(t + 1) * W], pts[t][:KM, :], KM)
                    # rows 0..487 = 4 tiles of 122 (fp16); SWDGE spreads
                    # the descriptors across all 16 SDMA engines
                    nc.gpsimd.dma_start(
                        bass.AP(out, p * H * W,
                                [[W, KM], [KM * W, 4], [1, W]]),
                        ob[:].rearrange("p (b w) -> p b w", b=4))

                # ---- merged bottom tiles of the group ----
                pt = pspool.tile([128, W], mybir.dt.float32, name="pt")
                xgt = xg[:].tensor
                for j, (off, dw0, dw1) in enumerate(PAIRS):
                    dr_matmul(pt, (4 + j) * 2 * M_PAD, xgt, 2 * GWB,
                              GWB, off, GROUP * 27, j)
                og = ogpool.tile([GROUP * 24, W], mybir.dt.float16)
                evict("act", og[:], pt[:GROUP * 24, :], GROUP * 24)
                for g in range(GROUP):
                    nc.gpsimd.dma_start(
                        bass.AP(out, ((g0 + g) * H + 488) * W,
                                [[W, 24], [1, W]]),
                        og[24 * g:24 * g + 24, :])

    nc.compile()
    _NC_CACHE[n_planes] = nc
    return nc


def _prep_inputs(X, K, b, n_cores=N_CORES):
    Keff = np.asarray(K, np.float32).sum(axis=(0, 1))
    K8 = Keff.astype(FP8).astype(np.float32)
    wt = _build_weight_pack(K8)
    bias = np.float32(np.asarray(b).reshape(-1)[0]) * np.float32(K.size)
    bv = np.full((128, 1), bias, np.float32)

    Xr = np.asarray(X, np.float32).reshape(-1, H, W)
    n_total = Xr.shape[0]
    per = n_total // n_cores
    # zero-padded fp8 planes (width WB) + one-column-left-shifted copy
    Xpad = np.zeros((n_total, H, WB), FP8)
    Xpad[:, :, 3:3 + W] = Xr.astype(FP8)
    Xsh = np.zeros((n_total, H, WB), FP8)
    Xsh[:, :, 0:WB - 1] = Xpad[:, :, 1:WB]
    # swizzle into partition-major [plane, partition, slot, block, WB]
    Xp = np.zeros((n_total, 128, 2, 4, WB), FP8)
    for s, src in ((0, Xpad), (1, Xsh)):
        Xp[:, 3:, s, 0, :] = src[:, 0:125, :]
        for bi, st in enumerate(BSTARTS):
            Xp[:, :, s, bi + 1, :] = src[:, st:st + 128, :]
    Xp = Xp.reshape(n_total, 128, 2 * SLOT)
    # group bottom blocks: [group, 4*27, 2, GWB], plane g at partitions 27g..
    Xg = np.zeros((n_total // GROUP, GROUP * 27, 2, GWB), FP8)
    Xg[:, :, 0, :WB] = Xpad[:, 485:512, :].reshape(-1, GROUP * 27, WB)
    Xg[:, :, 1, :WB] = Xsh[:, 485:512, :].reshape(-1, GROUP * 27, WB)
    Xg = Xg.reshape(n_total // GROUP, GROUP * 27, 2 * GWB)
    in_maps = [
        {"xp": Xp[i * per:(i + 1) * per],
         "xg": Xg[i * (per // GROUP):(i + 1) * (per // GROUP)],
         "wt": wt, "bv": bv}
        for i in range(n_cores)
    ]
    return in_maps, per


def kernel(X, K, b):
    in_maps, per = _prep_inputs(X, K, b)
    nc = _get_module(per)
    res = run_bass_kernel_spmd(nc, in_maps, list(range(N_CORES)))
    out = np.concatenate([res.results[i]["out"] for i in range(N_CORES)],
                         axis=0).astype(np.float32)
    return out.reshape(np.asarray(X).shape)


# revision 19
# speedup vs baseline: 1.7501x; 1.4093x over previous
"""Trainium2 Bass kernel for nn_Conv_39273180955616.

Computes, for X:(16,64,512,512) f32, K:(1,1,7,7), b:(1,1,1,1):
    out[n,c] = correlate2d(X[n,c], Keff, pad=3) + 49*b
where Keff = K.sum(axis=(0,1)).

Strategy: pure data parallel over the 1024 (n,c) planes -> 128 planes/core
on 8 cores.  Per plane, the 7x7 correlation runs on TensorE as
banded-Toeplitz matmuls in fp8(e4m3) DoubleRow mode: the h-dimension
contraction is a [128, 128] band matrix (7 diagonals of one kernel
column, dh = p - m) against an image block (rows on partitions), and the
7 w-shifts are free-dim offsets into a zero-padded image row.  DoubleRow
packs TWO w-shifts per matmul (2 fp8 MACs/cell/cycle): the rhs is a
3-dim AP [K, 2, W] whose pair dim selects between the image (slot 0) and
a one-column-left-shifted copy (slot 1, pair stride 2080 -- the HW
requires a multiple of 16), so 7 shifts take 4 matmuls: pairs
(0,1),(2,3),(4,5),(5,6), the last with zeroed slot-0 weights.  The host
pre-pads 3 zero rows into block 0 so all 4 row-blocks share one weight
kind.  The 24-row bottom tiles of 4 consecutive planes are packed into
one block-diagonal matmul set (stacked on partitions).

DMA: one partition-major fp8 load per plane ([128, 4160], SP ring,
contiguous 4KB descriptors); group bottom blocks on the ACT ring.
PSUM is evicted as fp16 with the bias added (alternating ScalarE /
VectorE) and stored via SWDGE so descriptors spread across all 16 SDMA
engines; the host upcasts to f32.
"""
import numpy as np
import ml_dtypes

import concourse.bass as bass
import concourse.tile as tile
from concourse import bacc, mybir
from concourse.bass_utils import run_bass_kernel_spmd

N_CORES = 8
H = 512
W = 512
WPAD = W + 6   # 3 zero columns each side (valid data span)
WB = 520       # per-block width in the swizzled layout (padded)
SLOT = 4 * WB  # 2080, pair stride for main tiles (multiple of 16)
GWB = 528      # xg per-slot width (multiple of 16)
N_PLANES_TOTAL = 16 * 64
PLANES_PER_CORE = N_PLANES_TOTAL // N_CORES  # 128
GROUP = 4  # planes per bottom-tile merge group
# input row start of blocks 1..3; block 0 is [3 zero rows, rows 0..124]
BSTARTS = (119, 241, 363)
# DoubleRow pairs: (rhs base offset, slot0 dw or None, slot1 dw=off+1)
PAIRS = [(0, 0, 1), (2, 2, 3), (4, 4, 5), (5, None, 6)]
KM = 122   # output rows per main tile
M_PAD = 128
WCOLS = 8 * 2 * M_PAD  # 4 main pairs + 4 group pairs, 2 slots each

FP8 = ml_dtypes.float8_e4m3


def _band(Keff, Kk, M, dw):
    """[Kk, M_PAD] band matrix: mat[p, m] = Keff[p - m, dw]."""
    mat = np.zeros((Kk, M_PAD), np.float32)
    p = np.arange(Kk)[:, None]
    m = np.arange(M)[None, :]
    dh = p - m
    ok = (dh >= 0) & (dh < 7)
    sub = np.zeros((Kk, M), np.float32)
    sub[ok] = Keff[dh[ok], dw]
    mat[:, :M] = sub
    return mat


def _build_weight_pack(K8: np.ndarray) -> np.ndarray:
    """K8 (7,7) f32 (already e4m3-rounded) -> [128, WCOLS] fp8 lhsT pairs.

    Main pair j at cols [j*256, +256): slot0 [0:128], slot1 [128:256].
    Group (block-diagonal, GROUP bottom tiles) pair j at [(4+j)*256, ...).
    """
    wp = np.zeros((128, WCOLS), np.float32)
    for j, (_, dw0, dw1) in enumerate(PAIRS):
        c0 = j * 2 * M_PAD
        if dw0 is not None:
            wp[:, c0:c0 + M_PAD] = _band(K8, 128, KM, dw0)
        wp[:, c0 + M_PAD:c0 + 2 * M_PAD] = _band(K8, 128, KM, dw1)
    # group kind: block-diagonal stack of GROUP (27 -> 24) bottom bands
    for j, (_, dw0, dw1) in enumerate(PAIRS):
        c0 = (4 + j) * 2 * M_PAD
        for s, dw in ((0, dw0), (1, dw1)):
            if dw is None:
                continue
            blk = _band(K8, 27, 24, dw)[:, :24]
            for g in range(GROUP):
                wp[27 * g:27 * g + 27,
                   c0 + s * M_PAD + 24 * g:c0 + s * M_PAD + 24 * g + 24] = blk
    return wp.astype(FP8)


_NC_CACHE = {}


def _get_module(n_planes: int):
    if n_planes in _NC_CACHE:
        return _NC_CACHE[n_planes]
    assert n_planes % GROUP == 0
    nc = bacc.Bacc("TRN2", target_bir_lowering=False, debug=False,
                   num_devices=N_CORES)
    xp = nc.dram_tensor("xp", [n_planes, 128, 2 * SLOT], mybir.dt.float8e4,
                        kind="ExternalInput")
    xg_d = nc.dram_tensor("xg", [n_planes // GROUP, GROUP * 27, 2 * GWB],
                          mybir.dt.float8e4, kind="ExternalInput")
    wt = nc.dram_tensor("wt", [128, WCOLS], mybir.dt.float8e4,
                        kind="ExternalInput")
    bv = nc.dram_tensor("bv", [128, 1], mybir.dt.float32,
                        kind="ExternalInput")
    # partition-major store layouts (host unshuffles): outm[p, r, b, w]
    # holds output row b*122+r; outb[p, r, w] holds row 488+r.
    outm = nc.dram_tensor("outm", [n_planes, KM, 4 * W], mybir.dt.float16,
                          kind="ExternalOutput")
    outb = nc.dram_tensor("outb", [n_planes, 24, W], mybir.dt.float16,
                          kind="ExternalOutput")

    x_elems = 128 * 2 * SLOT
    g_elems = GROUP * 27 * 2 * GWB
    DR = mybir.MatmulPerfMode.DoubleRow

    with tile.TileContext(nc) as tc:
        with (
            tc.tile_pool(name="wp", bufs=1) as wpool,
            tc.tile_pool(name="xa", bufs=8) as xapool,
            tc.tile_pool(name="xg", bufs=3) as xgpool,
            tc.tile_pool(name="ps", bufs=8, space="PSUM") as pspool,
            tc.tile_pool(name="ob", bufs=10) as obpool,
            tc.tile_pool(name="og", bufs=3) as ogpool,
        ):
            wtile = wpool.tile([128, WCOLS], mybir.dt.float8e4)
            nc.sync.dma_start(wtile[:], wt.ap())
            btile = wpool.tile([128, 1], mybir.dt.float32)
            nc.sync.dma_start(btile[:], bv.ap())

            def evict(engine, dst, src, rows):
                if engine == "act":
                    nc.scalar.activation(
                        dst, src, mybir.ActivationFunctionType.Identity,
                        bias=btile[:rows, :], scale=1.0)
                else:
                    nc.vector.tensor_scalar_add(dst, src, btile[:rows, :])

            def dr_matmul(pt, wcol0, tens, pstride, sstride, rhs_off, Kk, j):
                """One DoubleRow matmul: lhsT [K,2,128], rhs [K,2,512].

                rhs pair dim selects image slot 0 / shifted slot 1
                (sstride must be a multiple of 16 for DoubleRow).
                """
                lhsT = wtile[:Kk, wcol0:wcol0 + 2 * M_PAD].rearrange(
                    "k (two m) -> k two m", two=2)
                rhs = bass.AP(tens, rhs_off,
                              [[pstride, Kk], [sstride, 2], [1, W]])
                nc.tensor.matmul(pt[:, :], lhsT, rhs, start=(j == 0),
                                 stop=(j == 3), perf_mode=DR)

            for g0 in range(0, n_planes, GROUP):
                # bottom rows (485..511) of GROUP planes, pre-packed
                # block-diagonally on host; one load on the ACT ring
                xg = xgpool.tile([GROUP * 27, 2 * GWB], mybir.dt.float8e4)
                nc.scalar.dma_start(
                    xg[:], bass.AP(xg_d, (g0 // GROUP) * g_elems,
                                   [[2 * GWB, GROUP * 27], [1, 2 * GWB]]))
                for p in range(g0, g0 + GROUP):
                    # one partition-major load per plane (SP ring):
                    # partition r holds [4 blocks | 4 shifted blocks]
                    xa = xapool.tile([128, 2 * SLOT], mybir.dt.float8e4)
                    nc.sync.dma_start(
                        xa[:], bass.AP(xp, p * x_elems,
                                       [[2 * SLOT, 128], [1, 2 * SLOT]]))

                    ob = obpool.tile([122, 4 * W], mybir.dt.float16)
                    pts = [pspool.tile([128, W], mybir.dt.float32,
                                       name="pt")
                           for t in range(4)]
                    xat = xa[:].tensor
                    for j, (off, dw0, dw1) in enumerate(PAIRS):
                        for t in range(4):
                            dr_matmul(pts[t], j * 2 * M_PAD, xat, 2 * SLOT,
                                      SLOT, t * WB + off, 128, j)
                    for t in range(4):
                        evict("act" if t % 2 == 0 else "dve",
                              ob[:, t * W:(t + 1) * W], pts[t][:KM, :], KM)
                    # rows 0..487 = 4 tiles of 122 (fp16); partition-major
                    # DRAM layout -> 122 contiguous 4KB descriptors, SWDGE
                    # spreads them across all 16 SDMA engines
                    nc.gpsimd.dma_start(
                        bass.AP(outm, p * KM * 4 * W,
                                [[4 * W, KM], [1, 4 * W]]),
                        ob[:])

                # ---- merged bottom tiles of the group ----
                pt = pspool.tile([128, W], mybir.dt.float32, name="pt")
                xgt = xg[:].tensor
                for j, (off, dw0, dw1) in enumerate(PAIRS):
                    dr_matmul(pt, (4 + j) * 2 * M_PAD, xgt, 2 * GWB,
                              GWB, off, GROUP * 27, j)
                og = ogpool.tile([GROUP * 24, W], mybir.dt.float16)
                evict("act", og[:], pt[:GROUP * 24, :], GROUP * 24)
                for g in range(GROUP):
                    nc.scalar.dma_start(
                        bass.AP(outb, (g0 + g) * 24 * W,
                                [[W, 24], [1, W]]),
                        og[24 * g:24 * g + 24, :])

    nc.compile()
    _NC_CACHE[n_planes] = nc
    return nc


def _prep_inputs(X, K, b, n_cores=N_CORES):
    Keff = np.asarray(K, np.float32).sum(axis=(0, 1))
    K8 = Keff.astype(FP8).astype(np.float32)
    wt = _build_weight_pack(K8)
    bias = np.float32(np.asarray(b).reshape(-1)[0]) * np.float32(K.size)
    bv = np.full((128, 1), bias, np.float32)

    Xr = np.asarray(X, np.float32).reshape(-1, H, W)
    n_total = Xr.shape[0]
    per = n_total // n_cores
    # zero-padded fp8 planes (width WB) + one-column-left-shifted copy
    Xpad = np.zeros((n_total, H, WB), FP8)
    Xpad[:, :, 3:3 + W] = Xr.astype(FP8)
    Xsh = np.zeros((n_total, H, WB), FP8)
    Xsh[:, :, 0:WB - 1] = Xpad[:, :, 1:WB]
    # swizzle into partition-major [plane, partition, slot, block, WB]
    Xp = np.zeros((n_total, 128, 2, 4, WB), FP8)
    for s, src in ((0, Xpad), (1, Xsh)):
        Xp[:, 3:, s, 0, :] = src[:, 0:125, :]
        for bi, st in enumerate(BSTARTS):
            Xp[:, :, s, bi + 1, :] = src[:, st:st + 128, :]
    Xp = Xp.reshape(n_total, 128, 2 * SLOT)
    # group bottom blocks: [group, 4*27, 2, GWB], plane g at partitions 27g..
    Xg = np.zeros((n_total // GROUP, GROUP * 27, 2, GWB), FP8)
    Xg[:, :, 0, :WB] = Xpad[:, 485:512, :].reshape(-1, GROUP * 27, WB)
    Xg[:, :, 1, :WB] = Xsh[:, 485:512, :].reshape(-1, GROUP * 27, WB)
    Xg = Xg.reshape(n_total // GROUP, GROUP * 27, 2 * GWB)
    in_maps = [
        {"xp": Xp[i * per:(i + 1) * per],
         "xg": Xg[i * (per // GROUP):(i + 1) * (per // GROUP)],
         "wt": wt, "bv": bv}
        for i in range(n_cores)
    ]
    return in_maps, per


def kernel(X, K, b):
    in_maps, per = _prep_inputs(X, K, b)
    nc = _get_module(per)
    res = run_bass_kernel_spmd(nc, in_maps, list(range(N_CORES)))
    outm = np.concatenate([res.results[i]["outm"] for i in range(N_CORES)],
                          axis=0)  # [n, 122, 4*W] fp16
    outb = np.concatenate([res.results[i]["outb"] for i in range(N_CORES)],
                          axis=0)  # [n, 24, W] fp16
    n_total = outm.shape[0]
    full = np.empty((n_total, H, W), np.float32)
    # row b*122+r lives at outm[p, r, b*W:(b+1)*W]
    full[:, :4 * KM] = (outm.reshape(n_total, KM, 4, W)
                        .transpose(0, 2, 1, 3)
                        .reshape(n_total, 4 * KM, W).astype(np.float32))
    full[:, 4 * KM:] = outb.astype(np.float32)
    return full.reshape(np.asarray(X).shape)
